# revision 23
# baseline (speedup 1.0000x reference)
"""Trainium2 Bass kernel for nn_AxonalConnections.

Computes, per (batch b, patch n):
    out[t]  = sum_s sp[b,n,s] * W_dyn[b,n,t,s]          (batched matvec, distinct weights)
    out_n   = LayerNorm_T(out) * gamma + beta
    w       = softmax(out_n / TEMP)
    final   = w * (gates[n] * sum_s sp[b,n,s] + biases[n])
    fold -> [B, 256, 256]

Strategy: 8-way shard over (batch b, patch-half); each core owns 128 patches.

Key observation: source_spikes is binary with ~10% density, so the matvec is
a sum of the ~26 active s-columns of W per patch.  The host gathers exactly
those rows (W_dyn[b,n,:,s] for active s), packs them densely per 32-patch
window, and the device does the per-patch segmented sum on the TensorEngine:
each 128-row tile of packed rows is contracted against a [128 rows x 32
patches] 0/1 "staircase" membership mask (lhsT), accumulating into a
[32, 256] PSUM slice per window.  HBM traffic drops from 25.7 MB (dense
bf16+fp8 W stream) to ~2.3 MB (fp16 gathered rows + masks) per core.

Rows ship as fp16 (rel err ~1.6e-3 end to end, measured).  The LayerNorm +
temperature-softmax epilogue is fused: when gamma/beta are constant vectors
(they are: ones/zeros), logits - max(logits) == (x - max(x)) * s with
s = gamma0/(TEMP*std), so one ACT Exp op with per-partition scale/bias does
normalize+softmax straight out of PSUM.  rstd uses exp(-0.5*ln(var+eps)) so
only one ACT table set (natural_log_exp_and_others) is ever loaded.
Unfold/fold, the gather, and shard assembly are host-side numpy.
"""

import os
import sys

for _p in ("/opt/trn_rl_repo",):
    if _p not in sys.path:
        sys.path.insert(0, _p)

import numpy as np

import concourse.bass as bass
import concourse.bacc as bacc
import concourse.tile as tile
from concourse import mybir
from concourse import bass_utils

# Problem constants (hardcoded per contract)
B = 4
GRID = 256
PATCH = 16
PH = GRID // PATCH          # 16 patches per side
N = PH * PH                 # 256 patches
S = PATCH * PATCH           # 256 source pixels per patch
T = 256                     # 256 target pixels per patch
TEMP = 0.1
LN_EPS = 1e-5

NCORES = 8
P = 128                     # patches per core (= SBUF partitions)
NW = 4                      # windows of 32 patches (PSUM col-tiling granularity)
WPATCH = P // NW            # 32 patches per window

F32 = mybir.dt.float32
F16 = mybir.dt.float16
NWARM = int(os.environ.get("BASS_NWARM", "0"))

_NC_CACHE = {}


class _BaccOneActSet(bacc.Bacc):
    """Bacc whose act-table pass is forced to satisfy Ln AND Exp from the
    combined natural_log_exp_and_others set.  The default pass maps Ln ->
    natural_log and Exp -> exp_and_others, so a kernel alternating Ln/Exp
    reloads the ACT tables (~1.3us each) on every switch — 5 loads here,
    several on the critical path."""

    def insert_act_table_loads(self):
        from concourse.hw_specs import get_activation_tables
        from concourse.bacc import _bass_rust
        has_activation = any(
            isinstance(i, mybir.InstActivation)
            for b in self.main_func.blocks
            for i in b.instructions
        )
        if not has_activation:
            return
        both = {mybir.ActivationFunctionType.Ln,
                mybir.ActivationFunctionType.Exp}
        tables = []
        for name, funcs in get_activation_tables(self.m.arch).items():
            if name != "natural_log_exp_and_others":
                funcs = funcs - both
            tables.append((name, funcs))
        _bass_rust.insert_act_table_loads(self, tables)


def _wr_chunks(tiles):
    """Split each window's row-tiles into DMA chunks: small first chunk for a
    fast pipeline ramp, split last window so its tail DMA is short."""
    chunks = []
    for w, tw in enumerate(tiles):
        if w == 0 and tw > 4:
            chunks.append([3, tw - 3])
        elif w == len(tiles) - 1 and tw > 4:
            chunks.append([(tw + 1) // 2, tw // 2])
        else:
            chunks.append([tw])
    return chunks


def _build_nc(tiles, fast, ln_c):
    """tiles: per-window row-tile counts (same across cores); fast: constant
    gamma/beta epilogue; ln_c: log(gamma0/TEMP) for the fused scale."""
    nc = _BaccOneActSet("TRN2")
    G = sum(tiles)
    chunks = _wr_chunks(tiles)

    # packed gathered W rows: row r of tile g lives at [r%128, g*256 : +256]
    wr = nc.dram_tensor("wr", [P, G * T], F16, kind="ExternalInput")
    # staircase membership masks, one [128, 32] slab per row-tile
    mk = nc.dram_tensor("mk", [P, G * WPATCH], F16, kind="ExternalInput")
    # aux: [sp (S) | gate | bias], + [gamma/TEMP (T) | beta/TEMP (T)] general
    aux_w = (S + 2) if fast else (S + 2 + 2 * T)
    aux = nc.dram_tensor("aux", [P, aux_w], F32, kind="ExternalInput")
    outd = nc.dram_tensor("out", [P, T], F32, kind="ExternalOutput")

    Alu = mybir.AluOpType
    Act = mybir.ActivationFunctionType
    Ax = mybir.AxisListType

    HP = P // 2  # partitions per epilogue half (= 2 windows)

    with tile.TileContext(nc) as tc:
        with (
            tc.tile_pool(name="data", bufs=1) as data,
            tc.tile_pool(name="pspool", bufs=1, space="PSUM") as pspool,
            tc.tile_pool(name="small", bufs=2) as small,
        ):
            # ---- engine warmups first: the PE dummy stream needs its rhs
            # tile as the very first DVE op so matmuls start right after the
            # engine preamble and the HAM clock-gate lifts (1.2 -> 2.4 GHz)
            # before the real stream
            k_fold = float(np.exp(-2.0 * ln_c))
            eps_t = small.tile([P, 1], F32)
            nc.vector.memset(eps_t, LN_EPS * k_fold)
            if NWARM:
                wmt = small.tile([P, 2 * T], F16)
                nc.vector.memset(wmt, 0.0)
                wps = pspool.tile([P, 2 * T], F32, tag="warm")
                for _ in range(NWARM):
                    nc.tensor.matmul(wps[0:WPATCH, :], lhsT=wmt[:, 0:WPATCH],
                                     rhs=wmt, start=True, stop=True)

            # ---- DMA issue.  The ACT table load (~1.5us) is hoisted to the
            # top of the Scalar stream, so the scalar HWDGE ring only gets
            # late-needed data (masks for w1..w3, w3 rows); everything the
            # pipeline front needs goes on the Sync ring, aux on GpSimd SWDGE.
            wrt = {}          # (w, c) -> tile
            gg0 = {}          # (w, c) -> first global tile index
            g_run = 0
            for w, tw in enumerate(tiles):
                for c, cn in enumerate(chunks[w]):
                    wrt[(w, c)] = data.tile([P, cn, T], F16,
                                            name=f"wr{w}_{c}", tag=f"wr{w}_{c}")
                    gg0[(w, c)] = g_run
                    g_run += cn
            order_sync = [k for k in wrt if k[0] in (0, 1, 2)]
            order_scalar = [k for k in wrt if k[0] == 3]
            t0w = tiles[0]
            mka = data.tile([P, t0w, WPATCH], F16, tag="mka")
            mkb = data.tile([P, G - t0w, WPATCH], F16, tag="mkb")

            def _dma_wr(eng, key):
                g0, cn = gg0[key], chunks[key[0]][key[1]]
                eng.dma_start(
                    out=wrt[key].rearrange("p a b -> p (a b)"),
                    in_=wr[:, g0 * T : (g0 + cn) * T])

            nc.sync.dma_start(
                out=mka.rearrange("p a b -> p (a b)"),
                in_=mk[:, 0 : t0w * WPATCH])
            for key in order_sync:
                _dma_wr(nc.sync, key)
            nc.scalar.dma_start(
                out=mkb.rearrange("p a b -> p (a b)"),
                in_=mk[:, t0w * WPATCH :])
            for key in order_scalar:
                _dma_wr(nc.scalar, key)
            aux_t = data.tile([P, aux_w], F32, tag="aux")
            nc.gpsimd.dma_start(out=aux_t, in_=aux[:, :])
            spv_t = aux_t[:, 0:S]
            gat_t = aux_t[:, S : S + 1]
            bia_t = aux_t[:, S + 1 : S + 2]

            # per-patch scalar: gates * sum_s(sp) + biases, early
            spsum = small.tile([P, 1], F32)
            nc.vector.tensor_reduce(out=spsum, in_=spv_t, axis=Ax.X, op=Alu.add)
            scal2 = small.tile([P, 1], F32)
            nc.vector.tensor_scalar(out=scal2, in0=spsum, scalar1=gat_t,
                                    scalar2=bia_t, op0=Alu.mult, op1=Alu.add)

            # ---- epilogue tiles (full width; each half uses its slice) ----
            # separate PSUM tiles per half: a shared tile makes half 0's
            # epilogue reads a (coarse) WAR hazard for windows 2/3 matmuls
            ps01 = pspool.tile([P, T], F32, tag="acc01")
            ps23 = pspool.tile([P, T], F32, tag="acc23")
            psh = {0: ps01, 1: ps01, 2: ps23, 3: ps23}
            stats = small.tile([P, 6], F32)
            mv = small.tile([P, 2], F32)
            lnv = small.tile([P, 1], F32)
            sfac = small.tile([P, 1], F32)
            mx = small.tile([P, 1], F32)
            nb = small.tile([P, 1], F32)
            e = small.tile([P, T], F32)
            den = small.tile([P, 1], F32)
            rden = small.tile([P, 1], F32)
            fac = small.tile([P, 1], F32)
            fin = small.tile([P, T], F32)
            if not fast:
                z1 = small.tile([P, T], F32)
                z2 = small.tile([P, T], F32)
                z3 = small.tile([P, T], F32)

            def _epilogue(h):
                # fused LayerNorm + temperature softmax for partitions
                # [64h, 64h+64) — runs as soon as its two windows stop, so
                # half 0 overlaps the second half's matmul/DMA stream
                sl = slice(h * HP, (h + 1) * HP)
                ps = psh[2 * h]
                nc.vector.bn_stats(out=stats[sl, :], in_=ps[sl, :])
                nc.vector.bn_aggr(out=mv[sl, :], in_=stats[sl, :])
                nc.scalar.activation(out=lnv[sl, :], in_=mv[sl, 1:2],
                                     func=Act.Ln, scale=k_fold,
                                     bias=eps_t[sl, :])
                if fast:
                    # s = gamma0/(TEMP*std); logits - max == (x - mx)*s,
                    # so one Exp straight from PSUM does normalize+softmax
                    nc.scalar.activation(out=sfac[sl, :], in_=lnv[sl, :],
                                         func=Act.Exp, scale=-0.5)
                    nc.vector.tensor_reduce(out=mx[sl, :], in_=ps[sl, :],
                                            axis=Ax.X, op=Alu.max)
                    nc.vector.tensor_scalar(out=nb[sl, :], in0=mx[sl, :],
                                            scalar1=sfac[sl, :],
                                            scalar2=-1.0, op0=Alu.mult,
                                            op1=Alu.mult)
                    nc.scalar.activation(out=e[sl, :], in_=ps[sl, :],
                                         func=Act.Exp, bias=nb[sl, :],
                                         scale=sfac[sl, :],
                                         accum_out=den[sl, :])
                else:
                    nc.scalar.activation(out=sfac[sl, :], in_=lnv[sl, :],
                                         func=Act.Exp, scale=-0.5)
                    nc.vector.tensor_scalar(out=z1[sl, :], in0=ps[sl, :],
                                            scalar1=mv[sl, 0:1],
                                            scalar2=sfac[sl, :],
                                            op0=Alu.subtract, op1=Alu.mult)
                    nc.vector.tensor_mul(z2[sl, :], z1[sl, :],
                                         aux_t[sl, S + 2 : S + 2 + T])
                    nc.vector.tensor_add(z3[sl, :], z2[sl, :],
                                         aux_t[sl, S + 2 + T : S + 2 + 2 * T])
                    nc.vector.tensor_reduce(out=mx[sl, :], in_=z3[sl, :],
                                            axis=Ax.X, op=Alu.max)
                    nc.vector.tensor_scalar_mul(nb[sl, :], mx[sl, :], -1.0)
                    nc.scalar.activation(out=e[sl, :], in_=z3[sl, :],
                                         func=Act.Exp, bias=nb[sl, :],
                                         accum_out=den[sl, :])
                nc.vector.reciprocal(out=rden[sl, :], in_=den[sl, :])
                nc.vector.tensor_mul(fac[sl, :], scal2[sl, :], rden[sl, :])
                nc.vector.tensor_scalar_mul(fin[sl, :], e[sl, :], fac[sl, :])
                nc.sync.dma_start(out=outd[sl, :], in_=fin[sl, :])

            # ---- main pass: per-window segmented sums on the PE ----
            for w, tw in enumerate(tiles):
                g_base = sum(tiles[:w])
                g = 0
                for c, cn in enumerate(chunks[w]):
                    for i in range(cn):
                        gg = g_base + g
                        mk_sl = (mka[:, gg, :] if gg < t0w
                                 else mkb[:, gg - t0w, :])
                        nc.tensor.matmul(
                            psh[w][w * WPATCH : (w + 1) * WPATCH, :],
                            lhsT=mk_sl,
                            rhs=wrt[(w, c)][:, i, :],
                            start=(g == 0),
                            stop=(g == tw - 1),
                            tile_position=(0, w * WPATCH))
                        g += 1
                if w == 1:
                    _epilogue(0)
            _epilogue(1)
    nc.compile()
    return nc


def _get_nc(tiles, fast, ln_c):
    key = (tuple(tiles), fast, round(float(ln_c), 9))
    if key not in _NC_CACHE:
        _NC_CACHE[key] = _build_nc(list(tiles), fast, ln_c)
    return _NC_CACHE[key]


def _make_in_maps(source_spikes, W_dyn, ln_gamma, ln_beta, gates, biases):
    source_spikes = np.asarray(source_spikes, dtype=np.float32)
    W_dyn = np.asarray(W_dyn, dtype=np.float32)
    ln_gamma = np.asarray(ln_gamma, dtype=np.float32)
    ln_beta = np.asarray(ln_beta, dtype=np.float32)
    gates = np.asarray(gates, dtype=np.float32)
    biases = np.asarray(biases, dtype=np.float32)

    # unfold (matches reference._unfold with kernel=stride=16)
    sp_unf = (
        source_spikes.reshape(B, PH, PATCH, PH, PATCH)
        .transpose(0, 1, 3, 2, 4)
        .reshape(B, N, S)
    )
    sp_unf = np.ascontiguousarray(sp_unf)
    binary = bool(np.all((sp_unf == 0.0) | (sp_unf == 1.0)))

    # per-(core, window) active-row counts -> global per-window tile counts
    active = sp_unf != 0.0
    counts = active.sum(axis=2)                       # [B, N]
    rows_w = counts.reshape(B, 2, NW, WPATCH).sum(axis=3)   # [B, half, NW]
    tiles = [max(1, int(np.ceil(rows_w[:, :, w].max() / P)))
             for w in range(NW)]
    G = sum(tiles)

    fast = bool(
        np.all(ln_gamma == ln_gamma[0]) and np.all(ln_beta == ln_beta[0])
        and ln_gamma[0] > 0.0
    )
    ln_c = float(np.log(ln_gamma[0] / TEMP)) if fast else 0.0

    in_maps = []
    for c in range(NCORES):
        b, h = divmod(c, NCORES // B)
        n0 = h * P
        wrows = np.zeros((P, G * T), dtype=np.float16)
        masks = np.zeros((P, G * WPATCH), dtype=np.float16)
        g_base = 0
        for w in range(NW):
            tw = tiles[w]
            rw = tw * P
            rows = np.zeros((rw, T), dtype=np.float16)
            mrows = np.zeros((rw, WPATCH), dtype=np.float16)
            r = 0
            for j in range(WPATCH):
                n = n0 + w * WPATCH + j
                idx = np.nonzero(active[b, n])[0]
                k = idx.size
                if k:
                    blk = W_dyn[b, n][:, idx].T     # [k, T]
                    if not binary:
                        blk = blk * sp_unf[b, n, idx][:, None]
                    rows[r : r + k] = blk.astype(np.float16)
                    mrows[r : r + k, j] = 1.0
                    r += k
            # [tw*128, T] -> [128, tw, T] partition-major packing
            wrows[:, g_base * T : (g_base + tw) * T] = (
                rows.reshape(tw, P, T).transpose(1, 0, 2).reshape(P, tw * T))
            masks[:, g_base * WPATCH : (g_base + tw) * WPATCH] = (
                mrows.reshape(tw, P, WPATCH).transpose(1, 0, 2)
                .reshape(P, tw * WPATCH))
            g_base += tw

        aux_w = (S + 2) if fast else (S + 2 + 2 * T)
        aux = np.empty((P, aux_w), dtype=np.float32)
        aux[:, 0:S] = sp_unf[b, n0 : n0 + P]
        aux[:, S] = gates[n0 : n0 + P]
        aux[:, S + 1] = biases[n0 : n0 + P]
        if not fast:
            aux[:, S + 2 : S + 2 + T] = ln_gamma / TEMP
            aux[:, S + 2 + T :] = ln_beta / TEMP
        in_maps.append({
            "wr": wrows,
            "mk": masks,
            "aux": aux,
        })
    return in_maps, tiles, fast, ln_c


def _assemble(results):
    out_bnt = np.empty((B, N, T), dtype=np.float32)
    for c in range(NCORES):
        b, h = divmod(c, NCORES // B)
        n0 = h * P
        out_bnt[b, n0 : n0 + P] = results[c]["out"]
    # fold (matches reference._fold)
    return np.ascontiguousarray(
        out_bnt.reshape(B, PH, PH, PATCH, PATCH)
        .transpose(0, 1, 3, 2, 4)
        .reshape(B, GRID, GRID)
    )


def run_sharded(inputs: dict, trace: bool = False):
    """Run the SPMD bass kernel on 8 cores. Returns (output, BassKernelResults)."""
    in_maps, tiles, fast, ln_c = _make_in_maps(**inputs)
    nc = _get_nc(tiles, fast, ln_c)
    res = bass_utils.run_bass_kernel_spmd(nc, in_maps, list(range(NCORES)),
                                          trace=trace)
    return _assemble(res.results), res


def kernel(**inputs) -> np.ndarray:
    out, _ = run_sharded(inputs, trace=False)
    return out


# revision 26
# speedup vs baseline: 1.0371x; 1.0371x over previous
"""Trainium2 Bass kernel for nn_AxonalConnections.

Computes, per (batch b, patch n):
    out[t]  = sum_s sp[b,n,s] * W_dyn[b,n,t,s]          (batched matvec, distinct weights)
    out_n   = LayerNorm_T(out) * gamma + beta
    w       = softmax(out_n / TEMP)
    final   = w * (gates[n] * sum_s sp[b,n,s] + biases[n])
    fold -> [B, 256, 256]

Strategy: 8-way shard over (batch b, patch-half); each core owns 128 patches.

Key observation: source_spikes is binary with ~10% density, so the matvec is
a sum of the ~26 active s-columns of W per patch.  The host gathers exactly
those rows (W_dyn[b,n,:,s] for active s), packs them densely per 32-patch
window, and the device does the per-patch segmented sum on the TensorEngine:
each 128-row tile of packed rows is contracted against a [128 rows x 32
patches] 0/1 "staircase" membership mask (lhsT), accumulating into a
[32, 256] PSUM slice per window.  HBM traffic drops from 25.7 MB (dense
bf16+fp8 W stream) to ~2.3 MB (fp16 gathered rows + masks) per core.

Rows ship as fp16 (rel err ~1.6e-3 end to end, measured).  The LayerNorm +
temperature-softmax epilogue is fused: when gamma/beta are constant vectors
(they are: ones/zeros), logits - max(logits) == (x - max(x)) * s with
s = gamma0/(TEMP*std), so one ACT Exp op with per-partition scale/bias does
normalize+softmax straight out of PSUM.  rstd uses exp(-0.5*ln(var+eps)) so
only one ACT table set (natural_log_exp_and_others) is ever loaded.
Unfold/fold, the gather, and shard assembly are host-side numpy.
"""

import os
import sys

for _p in ("/opt/trn_rl_repo",):
    if _p not in sys.path:
        sys.path.insert(0, _p)

import numpy as np

import concourse.bass as bass
import concourse.bacc as bacc
import concourse.tile as tile
from concourse import mybir
from concourse import bass_utils

# Problem constants (hardcoded per contract)
B = 4
GRID = 256
PATCH = 16
PH = GRID // PATCH          # 16 patches per side
N = PH * PH                 # 256 patches
S = PATCH * PATCH           # 256 source pixels per patch
T = 256                     # 256 target pixels per patch
TEMP = 0.1
LN_EPS = 1e-5

NCORES = 8
P = 128                     # patches per core (= SBUF partitions)
NW = 4                      # windows of 32 patches (PSUM col-tiling granularity)
WPATCH = P // NW            # 32 patches per window

F32 = mybir.dt.float32
F16 = mybir.dt.float16
NWARM = int(os.environ.get("BASS_NWARM", "0"))

_NC_CACHE = {}


class _BaccOneActSet(bacc.Bacc):
    """Bacc whose act-table pass is forced to satisfy Ln AND Exp from the
    combined natural_log_exp_and_others set.  The default pass maps Ln ->
    natural_log and Exp -> exp_and_others, so a kernel alternating Ln/Exp
    reloads the ACT tables (~1.3us each) on every switch — 5 loads here,
    several on the critical path."""

    def insert_act_table_loads(self):
        from concourse.hw_specs import get_activation_tables
        from concourse.bacc import _bass_rust
        has_activation = any(
            isinstance(i, mybir.InstActivation)
            for b in self.main_func.blocks
            for i in b.instructions
        )
        if not has_activation:
            return
        both = {mybir.ActivationFunctionType.Ln,
                mybir.ActivationFunctionType.Exp}
        tables = []
        for name, funcs in get_activation_tables(self.m.arch).items():
            if name != "natural_log_exp_and_others":
                funcs = funcs - both
            tables.append((name, funcs))
        _bass_rust.insert_act_table_loads(self, tables)


def _wr_chunks(tiles):
    """Split each window's row-tiles into DMA chunks: small first chunk for a
    fast pipeline ramp, split last window so its tail DMA is short."""
    chunks = []
    for w, tw in enumerate(tiles):
        if w == 0 and tw > 4:
            chunks.append([3, tw - 3])
        elif w == len(tiles) - 1 and tw > 4:
            chunks.append([(tw + 1) // 2, tw // 2])
        else:
            chunks.append([tw])
    return chunks


def _build_nc(tiles, fast, ln_c):
    """tiles: per-window row-tile counts (same across cores); fast: constant
    gamma/beta epilogue; ln_c: log(gamma0/TEMP) for the fused scale."""
    nc = _BaccOneActSet("TRN2")
    G = sum(tiles)
    chunks = _wr_chunks(tiles)

    # packed gathered W rows: row r of tile g lives at [r%128, g*256 : +256]
    wr = nc.dram_tensor("wr", [P, G * T], F16, kind="ExternalInput")
    # staircase membership masks, one [128, 32] slab per row-tile
    mk = nc.dram_tensor("mk", [P, G * WPATCH], F16, kind="ExternalInput")
    # aux: [sp (S) | gate | bias], + [gamma/TEMP (T) | beta/TEMP (T)] general
    aux_w = (S + 2) if fast else (S + 2 + 2 * T)
    aux = nc.dram_tensor("aux", [P, aux_w], F32, kind="ExternalInput")
    outd = nc.dram_tensor("out", [P, T], F32, kind="ExternalOutput")

    Alu = mybir.AluOpType
    Act = mybir.ActivationFunctionType
    Ax = mybir.AxisListType

    HP = P // 2  # partitions per epilogue half (= 2 windows)

    with tile.TileContext(nc) as tc:
        with (
            tc.tile_pool(name="data", bufs=1) as data,
            tc.tile_pool(name="pspool", bufs=1, space="PSUM") as pspool,
            tc.tile_pool(name="small", bufs=2) as small,
        ):
            # ---- engine warmups first: the PE dummy stream needs its rhs
            # tile as the very first DVE op so matmuls start right after the
            # engine preamble and the HAM clock-gate lifts (1.2 -> 2.4 GHz)
            # before the real stream
            k_fold = float(np.exp(-2.0 * ln_c))
            eps_t = small.tile([P, 1], F32)
            nc.vector.memset(eps_t, LN_EPS * k_fold)
            if NWARM:
                wmt = small.tile([P, 2 * T], F16)
                nc.vector.memset(wmt, 0.0)
                wps = pspool.tile([P, 2 * T], F32, tag="warm")
                for _ in range(NWARM):
                    nc.tensor.matmul(wps[0:WPATCH, :], lhsT=wmt[:, 0:WPATCH],
                                     rhs=wmt, start=True, stop=True)

            # ---- DMA issue.  The ACT table load (~1.3us) is hoisted to the
            # top of the Scalar stream, so the scalar HWDGE ring only gets
            # late-needed data (masks for w1..w3, w2/w3 rows, aux); everything
            # the pipeline front needs goes on the Sync ring.  (GpSimd SWDGE
            # is avoided entirely: its descriptor generation runs ~5us on the
            # Q7 and measurably throttles the SDMA engines while it runs.)
            wrt = {}          # (w, c) -> tile
            gg0 = {}          # (w, c) -> first global tile index
            g_run = 0
            for w, tw in enumerate(tiles):
                for c, cn in enumerate(chunks[w]):
                    wrt[(w, c)] = data.tile([P, cn, T], F16,
                                            name=f"wr{w}_{c}", tag=f"wr{w}_{c}")
                    gg0[(w, c)] = g_run
                    g_run += cn
            order_sync = [k for k in wrt if k[0] in (0, 1)]
            order_scalar = [k for k in wrt if k[0] in (2, 3)]
            t0w = tiles[0]
            mka = data.tile([P, t0w, WPATCH], F16, tag="mka")
            mkb = data.tile([P, G - t0w, WPATCH], F16, tag="mkb")

            def _dma_wr(eng, key):
                g0, cn = gg0[key], chunks[key[0]][key[1]]
                eng.dma_start(
                    out=wrt[key].rearrange("p a b -> p (a b)"),
                    in_=wr[:, g0 * T : (g0 + cn) * T])

            nc.sync.dma_start(
                out=mka.rearrange("p a b -> p (a b)"),
                in_=mk[:, 0 : t0w * WPATCH])
            for key in order_sync:
                _dma_wr(nc.sync, key)
            nc.scalar.dma_start(
                out=mkb.rearrange("p a b -> p (a b)"),
                in_=mk[:, t0w * WPATCH :])
            for key in order_scalar:
                _dma_wr(nc.scalar, key)
            aux_t = data.tile([P, aux_w], F32, tag="aux")
            nc.scalar.dma_start(out=aux_t, in_=aux[:, :])
            spv_t = aux_t[:, 0:S]
            gat_t = aux_t[:, S : S + 1]
            bia_t = aux_t[:, S + 1 : S + 2]

            # per-patch scalar: gates * sum_s(sp) + biases, early
            spsum = small.tile([P, 1], F32)
            nc.vector.tensor_reduce(out=spsum, in_=spv_t, axis=Ax.X, op=Alu.add)
            scal2 = small.tile([P, 1], F32)
            nc.vector.tensor_scalar(out=scal2, in0=spsum, scalar1=gat_t,
                                    scalar2=bia_t, op0=Alu.mult, op1=Alu.add)

            # ---- epilogue tiles (full width; each half uses its slice) ----
            # separate PSUM tiles per half: a shared tile makes half 0's
            # epilogue reads a (coarse) WAR hazard for windows 2/3 matmuls
            ps01 = pspool.tile([P, T], F32, tag="acc01")
            ps23 = pspool.tile([P, T], F32, tag="acc23")
            psh = {0: ps01, 1: ps01, 2: ps23, 3: ps23}
            stats = small.tile([P, 6], F32)
            mv = small.tile([P, 2], F32)
            lnv = small.tile([P, 1], F32)
            sfac = small.tile([P, 1], F32)
            mx = small.tile([P, 1], F32)
            nb = small.tile([P, 1], F32)
            e = small.tile([P, T], F32)
            den = small.tile([P, 1], F32)
            rden = small.tile([P, 1], F32)
            fac = small.tile([P, 1], F32)
            fin = small.tile([P, T], F32)
            if not fast:
                z1 = small.tile([P, T], F32)
                z2 = small.tile([P, T], F32)
                z3 = small.tile([P, T], F32)

            def _epilogue(h):
                # fused LayerNorm + temperature softmax for partitions
                # [64h, 64h+64) — runs as soon as its two windows stop, so
                # half 0 overlaps the second half's matmul/DMA stream
                sl = slice(h * HP, (h + 1) * HP)
                ps = psh[2 * h]
                nc.vector.bn_stats(out=stats[sl, :], in_=ps[sl, :])
                nc.vector.bn_aggr(out=mv[sl, :], in_=stats[sl, :])
                nc.scalar.activation(out=lnv[sl, :], in_=mv[sl, 1:2],
                                     func=Act.Ln, scale=k_fold,
                                     bias=eps_t[sl, :])
                if fast:
                    # s = gamma0/(TEMP*std); logits - max == (x - mx)*s,
                    # so one Exp straight from PSUM does normalize+softmax
                    nc.scalar.activation(out=sfac[sl, :], in_=lnv[sl, :],
                                         func=Act.Exp, scale=-0.5)
                    nc.vector.tensor_reduce(out=mx[sl, :], in_=ps[sl, :],
                                            axis=Ax.X, op=Alu.max)
                    nc.vector.tensor_scalar(out=nb[sl, :], in0=mx[sl, :],
                                            scalar1=sfac[sl, :],
                                            scalar2=-1.0, op0=Alu.mult,
                                            op1=Alu.mult)
                    nc.scalar.activation(out=e[sl, :], in_=ps[sl, :],
                                         func=Act.Exp, bias=nb[sl, :],
                                         scale=sfac[sl, :],
                                         accum_out=den[sl, :])
                else:
                    nc.scalar.activation(out=sfac[sl, :], in_=lnv[sl, :],
                                         func=Act.Exp, scale=-0.5)
                    nc.vector.tensor_scalar(out=z1[sl, :], in0=ps[sl, :],
                                            scalar1=mv[sl, 0:1],
                                            scalar2=sfac[sl, :],
                                            op0=Alu.subtract, op1=Alu.mult)
                    nc.vector.tensor_mul(z2[sl, :], z1[sl, :],
                                         aux_t[sl, S + 2 : S + 2 + T])
                    nc.vector.tensor_add(z3[sl, :], z2[sl, :],
                                         aux_t[sl, S + 2 + T : S + 2 + 2 * T])
                    nc.vector.tensor_reduce(out=mx[sl, :], in_=z3[sl, :],
                                            axis=Ax.X, op=Alu.max)
                    nc.vector.tensor_scalar_mul(nb[sl, :], mx[sl, :], -1.0)
                    nc.scalar.activation(out=e[sl, :], in_=z3[sl, :],
                                         func=Act.Exp, bias=nb[sl, :],
                                         accum_out=den[sl, :])
                nc.vector.reciprocal(out=rden[sl, :], in_=den[sl, :])
                nc.vector.tensor_mul(fac[sl, :], scal2[sl, :], rden[sl, :])
                nc.vector.tensor_scalar_mul(fin[sl, :], e[sl, :], fac[sl, :])
                nc.sync.dma_start(out=outd[sl, :], in_=fin[sl, :])

            # ---- main pass: per-window segmented sums on the PE ----
            for w, tw in enumerate(tiles):
                g_base = sum(tiles[:w])
                g = 0
                for c, cn in enumerate(chunks[w]):
                    for i in range(cn):
                        gg = g_base + g
                        mk_sl = (mka[:, gg, :] if gg < t0w
                                 else mkb[:, gg - t0w, :])
                        nc.tensor.matmul(
                            psh[w][w * WPATCH : (w + 1) * WPATCH, :],
                            lhsT=mk_sl,
                            rhs=wrt[(w, c)][:, i, :],
                            start=(g == 0),
                            stop=(g == tw - 1),
                            tile_position=(0, w * WPATCH))
                        g += 1
                if w == 1:
                    _epilogue(0)
            _epilogue(1)
    nc.compile()
    return nc


def _get_nc(tiles, fast, ln_c):
    key = (tuple(tiles), fast, round(float(ln_c), 9))
    if key not in _NC_CACHE:
        _NC_CACHE[key] = _build_nc(list(tiles), fast, ln_c)
    return _NC_CACHE[key]


def _make_in_maps(source_spikes, W_dyn, ln_gamma, ln_beta, gates, biases):
    source_spikes = np.asarray(source_spikes, dtype=np.float32)
    W_dyn = np.asarray(W_dyn, dtype=np.float32)
    ln_gamma = np.asarray(ln_gamma, dtype=np.float32)
    ln_beta = np.asarray(ln_beta, dtype=np.float32)
    gates = np.asarray(gates, dtype=np.float32)
    biases = np.asarray(biases, dtype=np.float32)

    # unfold (matches reference._unfold with kernel=stride=16)
    sp_unf = (
        source_spikes.reshape(B, PH, PATCH, PH, PATCH)
        .transpose(0, 1, 3, 2, 4)
        .reshape(B, N, S)
    )
    sp_unf = np.ascontiguousarray(sp_unf)
    binary = bool(np.all((sp_unf == 0.0) | (sp_unf == 1.0)))

    # per-(core, window) active-row counts -> global per-window tile counts
    active = sp_unf != 0.0
    counts = active.sum(axis=2)                       # [B, N]
    rows_w = counts.reshape(B, 2, NW, WPATCH).sum(axis=3)   # [B, half, NW]
    tiles = [max(1, int(np.ceil(rows_w[:, :, w].max() / P)))
             for w in range(NW)]
    G = sum(tiles)

    fast = bool(
        np.all(ln_gamma == ln_gamma[0]) and np.all(ln_beta == ln_beta[0])
        and ln_gamma[0] > 0.0
    )
    ln_c = float(np.log(ln_gamma[0] / TEMP)) if fast else 0.0

    in_maps = []
    for c in range(NCORES):
        b, h = divmod(c, NCORES // B)
        n0 = h * P
        wrows = np.zeros((P, G * T), dtype=np.float16)
        masks = np.zeros((P, G * WPATCH), dtype=np.float16)
        g_base = 0
        for w in range(NW):
            tw = tiles[w]
            rw = tw * P
            rows = np.zeros((rw, T), dtype=np.float16)
            mrows = np.zeros((rw, WPATCH), dtype=np.float16)
            r = 0
            for j in range(WPATCH):
                n = n0 + w * WPATCH + j
                idx = np.nonzero(active[b, n])[0]
                k = idx.size
                if k:
                    blk = W_dyn[b, n][:, idx].T     # [k, T]
                    if not binary:
                        blk = blk * sp_unf[b, n, idx][:, None]
                    rows[r : r + k] = blk.astype(np.float16)
                    mrows[r : r + k, j] = 1.0
                    r += k
            # [tw*128, T] -> [128, tw, T] partition-major packing
            wrows[:, g_base * T : (g_base + tw) * T] = (
                rows.reshape(tw, P, T).transpose(1, 0, 2).reshape(P, tw * T))
            masks[:, g_base * WPATCH : (g_base + tw) * WPATCH] = (
                mrows.reshape(tw, P, WPATCH).transpose(1, 0, 2)
                .reshape(P, tw * WPATCH))
            g_base += tw

        aux_w = (S + 2) if fast else (S + 2 + 2 * T)
        aux = np.empty((P, aux_w), dtype=np.float32)
        aux[:, 0:S] = sp_unf[b, n0 : n0 + P]
        aux[:, S] = gates[n0 : n0 + P]
        aux[:, S + 1] = biases[n0 : n0 + P]
        if not fast:
            aux[:, S + 2 : S + 2 + T] = ln_gamma / TEMP
            aux[:, S + 2 + T :] = ln_beta / TEMP
        in_maps.append({
            "wr": wrows,
            "mk": masks,
            "aux": aux,
        })
    return in_maps, tiles, fast, ln_c


def _assemble(results):
    out_bnt = np.empty((B, N, T), dtype=np.float32)
    for c in range(NCORES):
        b, h = divmod(c, NCORES // B)
        n0 = h * P
        out_bnt[b, n0 : n0 + P] = results[c]["out"]
    # fold (matches reference._fold)
    return np.ascontiguousarray(
        out_bnt.reshape(B, PH, PH, PATCH, PATCH)
        .transpose(0, 1, 3, 2, 4)
        .reshape(B, GRID, GRID)
    )


def run_sharded(inputs: dict, trace: bool = False):
    """Run the SPMD bass kernel on 8 cores. Returns (output, BassKernelResults)."""
    in_maps, tiles, fast, ln_c = _make_in_maps(**inputs)
    nc = _get_nc(tiles, fast, ln_c)
    res = bass_utils.run_bass_kernel_spmd(nc, in_maps, list(range(NCORES)),
                                          trace=trace)
    return _assemble(res.results), res


def kernel(**inputs) -> np.ndarray:
    out, _ = run_sharded(inputs, trace=False)
    return out
